# revision 14
# baseline (speedup 1.0000x reference)
"""ContraNorm Trainium2 kernel (8 NeuronCores, flash-style, no NxN materialization).

Reference computation (N=16384, D=256, f32):
    x_norm = x / max(||x||_row, 1e-12)
    sim    = softmax(x_norm @ x_norm.T, axis=1)
    out    = 1.1 * x - 0.1 * (sim @ x)

Sharding: row-parallel. Every core receives the FULL x plus its own 2048-row
slice xr; core c computes output rows [c*2048, (c+1)*2048). No collectives.

Per-core algorithm (matmul operands fp16, accumulation fp32 in PSUM):
  setup:
    ssq[n]  = sum_d x[n,d]^2                  (DVE tensor_tensor_reduce)
    rnorm   = 1/sqrt(ssq)                     (ACT sqrt + DVE reciprocal)
    xa      = [fp16(x) | 1.0]  natural layout (GPSIMD copy, [128, 257] per chunk)
    xn      = fp16(x) * rnorm  per chunk      (DVE tensor_scalar)
    xnT     = transpose(xn)   [256, N]        (DMA xbar transpose, fp16)
    xnTm    = transpose of core's normalized rows [256, M]
    xrs     = 1.1 * xr (f32, resident)
  main (flash-style; cosine sims bounded in [-1,1] => no softmax max-subtraction):
    for each m-tile (512 rows) x n-chunk (128 rows):
      S_T[n,m] = xnT_chunk.T @ xnTm_tile     (PE, PSUM f32)
      E = exp(S_T)                           (ACT, fp16 out, [128,1024] insts)
      Paug[m, 0:257] += E_sub.T @ [x | 1]    (PE accumulate; col 256 = sum(exp))
    out_rows = xrs - 0.1 * Paug[:, :256] / Paug[:, 256]
"""

import numpy as np

N, D, NCORES = 16384, 256, 8
M = N // NCORES          # 2048 rows per core
P = 128                  # partitions
SCALE = 0.1

_NC = None               # cached compiled Bass module


def build(n=N, m=M, compile=True):
    """Build the per-core Bass program for full-row-count n, own-rows m."""
    import concourse.bass as bass
    import concourse.tile as tile
    from concourse import bacc, mybir
    from contextlib import ExitStack

    F16 = mybir.dt.float16
    F32 = mybir.dt.float32
    AF = mybir.ActivationFunctionType

    nch = n // P             # n-chunks
    rch = m // P             # own row-chunks
    mt_w = min(512, m)       # m-tile width (S matmul free dim)
    nmt = m // mt_w          # m-tiles
    msub = mt_w // P         # 128-row subtiles per m-tile

    nc = bacc.Bacc("TRN2", debug=False, num_devices=NCORES)
    x_d = nc.dram_tensor("x", (n, D), F32, kind="ExternalInput").ap()
    xr_d = nc.dram_tensor("xr", (m, D), F32, kind="ExternalInput").ap()
    out_d = nc.dram_tensor("out", (m, D), F32, kind="ExternalOutput").ap()

    # DRAM views: [partition, chunk, d]
    x_c = x_d.rearrange("(c p) d -> p c d", p=P)
    xr_c = xr_d.rearrange("(c p) d -> p c d", p=P)
    out_c = out_d.rearrange("(c p) d -> p c d", p=P)

    with tile.TileContext(nc) as tc, ExitStack() as ctx:
        big = ctx.enter_context(tc.tile_pool(name="big", bufs=1))
        ld = ctx.enter_context(tc.tile_pool(name="ld", bufs=2))
        sc_p = ctx.enter_context(tc.tile_pool(name="scr", bufs=2))
        xnp = ctx.enter_context(tc.tile_pool(name="xn", bufs=3))
        ep = ctx.enter_context(tc.tile_pool(name="exp", bufs=3))
        fin = ctx.enter_context(tc.tile_pool(name="fin", bufs=2))
        sp = ctx.enter_context(tc.tile_pool(name="spsum", bufs=2, space="PSUM"))
        pp = ctx.enter_context(tc.tile_pool(name="ppsum", bufs=1, space="PSUM"))

        # persistent tiles
        xa = big.tile([P, nch, 257], F16)          # raw x fp16 + ones col
        xnT0 = big.tile([P, n], F16)               # x_norm^T rows d=0..127
        xnT1 = big.tile([P, n], F16)               # x_norm^T rows d=128..255
        xnTm0 = big.tile([P, m], F16)              # core rows, normalized, transposed
        xnTm1 = big.tile([P, m], F16)
        xrs = big.tile([P, rch, D], F32)           # 1.1 * xr
        xrf = big.tile([P, rch, D], F16)           # fp16 copy of xr
        ssq_all = big.tile([P, nch], F32)
        rnorm_all = big.tile([P, nch], F32)
        ssq_r = big.tile([P, rch], F32)
        rnorm_r = big.tile([P, rch], F32)

        G = 4  # chunks per load group

        # ---- setup: core's own rows FIRST (they gate the very first matmul) ----
        for g in range(max(1, rch // G)):
            gw = min(G, rch)
            xt = ld.tile([P, G, D], F32)
            nc.sync.dma_start(xt[:, 0:gw, :], xr_c[:, g * gw:(g + 1) * gw, :])
            nc.vector.tensor_scalar_mul(xrs[:, g * gw:(g + 1) * gw, :],
                                        xt[:, 0:gw, :], 1.1)
            for j in range(gw):
                c = g * gw + j
                scr = sc_p.tile([P, D], F16, tag="sq_scratch")
                # row sum-of-squares via ACT Square + accum (TTR/tensor_reduce
                # are broken on this runtime)
                nc.scalar.activation(scr[:], xt[:, j, :], AF.Square,
                                     accum_out=ssq_r[:, c:c + 1])
                nc.gpsimd.tensor_copy(xrf[:, c, :], xt[:, j, :])
            s = sc_p.tile([P, gw], F32, tag="nrm_scratch")
            nc.scalar.sqrt(s[:], ssq_r[:, g * gw:(g + 1) * gw])
            nc.vector.reciprocal(rnorm_r[:, g * gw:(g + 1) * gw], s[:])
            for j in range(gw):
                c = g * gw + j
                xn = xnp.tile([P, D], F16)
                nc.vector.tensor_scalar_mul(xn[:], xrf[:, c, :], rnorm_r[:, c:c + 1])
                # split transposes across both HWDGE engines (2x throughput)
                nc.sync.dma_start_transpose(xnTm0[:, c * P:(c + 1) * P], xn[:, 0:P])
                nc.sync.dma_start_transpose(xnTm1[:, c * P:(c + 1) * P], xn[:, P:D])

        # ---- setup: full x, group-interleaved so chunk c is ready early ----
        for g in range(nch // G):
            xt = ld.tile([P, G, D], F32)
            nc.sync.dma_start(xt[:], x_c[:, g * G:(g + 1) * G, :])
            for j in range(G):
                c = g * G + j
                scr = sc_p.tile([P, D], F16, tag="sq_scratch")
                nc.scalar.activation(scr[:], xt[:, j, :], AF.Square,
                                     accum_out=ssq_all[:, c:c + 1])
                # raw fp16 copy (gpsimd is otherwise idle) + ones column
                nc.gpsimd.tensor_copy(xa[:, c, 0:D], xt[:, j, :])
                nc.gpsimd.memset(xa[:, c, D:257], 1.0)
            s = sc_p.tile([P, G], F32, tag="nrm_scratch")
            nc.scalar.sqrt(s[:], ssq_all[:, g * G:(g + 1) * G])
            nc.vector.reciprocal(rnorm_all[:, g * G:(g + 1) * G], s[:])
            for j in range(G):
                c = g * G + j
                # normalize from resident fp16 copy (no second DMA pass over x)
                xn = xnp.tile([P, D], F16)
                nc.vector.tensor_scalar_mul(xn[:], xa[:, c, 0:D],
                                            rnorm_all[:, c:c + 1])
                nc.sync.dma_start_transpose(xnT0[:, c * P:(c + 1) * P], xn[:, 0:P])
                nc.sync.dma_start_transpose(xnT1[:, c * P:(c + 1) * P], xn[:, P:D])

        # ---- main loop: n-phased so PE tracks the transpose stream ----
        # P accumulates in PSUM over CP-chunk blocks, then flushes into SBUF
        # accumulators (pacc); all m-tiles consume each chunk block while the
        # next block's transposes are still in flight.
        CP = min(8, nch)                   # chunks per phase
        nph = nch // CP
        pacc = big.tile([P, nmt * msub, 257], F32)   # SBUF P/sumexp accumulators
        for ph in range(nph):
            for mt in range(nmt):
                m0 = mt * mt_w
                paug = [pp.tile([P, 257], F32, tag=f"paug{ms}",
                                name=f"paug{ms}_{ph}_{mt}") for ms in range(msub)]
                for scn in range(CP // 2):
                    c0 = ph * CP + scn * 2
                    stp = sp.tile([P, 2, mt_w], F32)   # 2 n-chunks, 2 PSUM banks
                    for j in range(2):
                        c = c0 + j
                        nc.tensor.matmul(stp[:, j, :], xnT0[:, c * P:(c + 1) * P],
                                         xnTm0[:, m0:m0 + mt_w],
                                         start=True, stop=False)
                        nc.tensor.matmul(stp[:, j, :], xnT1[:, c * P:(c + 1) * P],
                                         xnTm1[:, m0:m0 + mt_w],
                                         start=False, stop=True)
                    es = ep.tile([P, 2, mt_w], F16)
                    nc.scalar.activation(es[:], stp[:], AF.Exp)
                    for j in range(2):
                        c = c0 + j
                        first = (c == ph * CP)
                        last = (c == ph * CP + CP - 1)
                        for ms in range(msub):
                            nc.tensor.matmul(
                                paug[ms][:], es[:, j, ms * P:(ms + 1) * P],
                                xa[:, c, :], start=first, stop=last,
                            )
                for ms in range(msub):
                    acc = pacc[:, mt * msub + ms, :]
                    if ph == 0:
                        nc.vector.tensor_copy(acc, paug[ms][:])
                    else:
                        nc.vector.tensor_add(acc, acc, paug[ms][:])

        # ---- finalize from SBUF accumulators ----
        for rc in range(nmt * msub):
            r = fin.tile([P, 1], F32, tag="recip")
            nc.vector.reciprocal(r[:], pacc[:, rc, 256:257])
            rs = fin.tile([P, 1], F32, tag="rscaled")
            nc.vector.tensor_scalar_mul(rs[:], r[:], -SCALE)
            t1 = fin.tile([P, D], F32, tag="scaledP")
            nc.vector.tensor_scalar_mul(t1[:], pacc[:, rc, 0:D], rs[:])
            ot = fin.tile([P, D], F32, tag="otile")
            nc.vector.tensor_add(ot[:], xrs[:, rc, :], t1[:])
            nc.sync.dma_start(out_c[:, rc, :], ot[:])

    if compile:
        nc.compile()
    return nc


def _get_nc():
    global _NC
    if _NC is None:
        _NC = build()
    return _NC


def _run(x, trace=False):
    from concourse.bass_utils import run_bass_kernel_spmd

    x = np.ascontiguousarray(np.asarray(x, dtype=np.float32))
    assert x.shape == (N, D)
    in_maps = [{"x": x, "xr": x[c * M:(c + 1) * M]} for c in range(NCORES)]
    res = run_bass_kernel_spmd(_get_nc(), in_maps, core_ids=list(range(NCORES)),
                               trace=trace)
    out = np.concatenate([res.results[c]["out"] for c in range(NCORES)], axis=0)
    return out, res


def kernel(x):
    out, _ = _run(x, trace=False)
    return out


# revision 15
# speedup vs baseline: 1.2312x; 1.2312x over previous
"""ContraNorm Trainium2 kernel (8 NeuronCores, flash-style, no NxN materialization).

Reference computation (N=16384, D=256, f32):
    x_norm = x / max(||x||_row, 1e-12)
    sim    = softmax(x_norm @ x_norm.T, axis=1)
    out    = 1.1 * x - 0.1 * (sim @ x)

Sharding: row-parallel. Every core receives the FULL x plus its own 2048-row
slice xr; core c computes output rows [c*2048, (c+1)*2048). No collectives.

Per-core algorithm (matmul operands fp16, accumulation fp32 in PSUM):
  setup (per 4-chunk group, software-pipelined with the main loop):
    ssq[n]  = sum_d x[n,d]^2                  (ACT Square + accum_out)
    rnorm   = 1/sqrt(ssq)                     (ACT sqrt + DVE reciprocal)
    xa      = [fp16(x) | 1.0]  natural layout (GPSIMD copy + memset)
    xn      = fp16(x) * rnorm                 (DVE tensor_scalar)
    xnT     = transpose(xn)   [256, N]        (DMA xbar transpose on Sync)
  main (flash-style; cosine sims bounded in [-1,1] => no max-subtraction):
    phased over n so PE tracks the transpose stream; for each 8-chunk phase,
    each 512-row m-tile computes
      S_T[n,m] = xnT_chunk.T @ xnTm_tile     (PE, PSUM f32)
      E = exp(S_T)                           (ACT, fp16 out, [128,1024] insts)
      Paug[m, 0:257] += E_sub.T @ [x | 1]    (PE accumulate; col 256 = sum(exp))
    then flushes Paug into SBUF accumulators (DVE adds).
  finalize: out_rows = 1.1*xr - 0.1 * Pacc[:, :256] / Pacc[:, 256]
"""

import numpy as np

N, D, NCORES = 16384, 256, 8
M = N // NCORES          # 2048 rows per core
P = 128                  # partitions
SCALE = 0.1

_NC = None               # cached compiled Bass module


def build(n=N, m=M, compile=True):
    """Build the per-core Bass program for full-row-count n, own-rows m."""
    import concourse.bass as bass
    import concourse.tile as tile
    from concourse import bacc, mybir
    from contextlib import ExitStack

    F16 = mybir.dt.float16
    F32 = mybir.dt.float32
    AF = mybir.ActivationFunctionType

    nch = n // P             # n-chunks
    rch = m // P             # own row-chunks
    mt_w = min(512, m)       # m-tile width (S matmul free dim)
    nmt = m // mt_w          # m-tiles
    msub = mt_w // P         # 128-row subtiles per m-tile
    G = 4                    # chunks per setup group
    ngrp = nch // G
    CP = min(8, nch)         # chunks per main-loop phase
    nph = nch // CP
    GPP = CP // G            # setup groups per phase
    LOOKAHEAD = 2            # phases of setup emitted ahead of compute

    nc = bacc.Bacc("TRN2", debug=False, num_devices=NCORES)
    x_d = nc.dram_tensor("x", (n, D), F32, kind="ExternalInput").ap()
    xr_d = nc.dram_tensor("xr", (m, D), F32, kind="ExternalInput").ap()
    out_d = nc.dram_tensor("out", (m, D), F32, kind="ExternalOutput").ap()

    # DRAM views: [partition, chunk, d]
    x_c = x_d.rearrange("(c p) d -> p c d", p=P)
    xr_c = xr_d.rearrange("(c p) d -> p c d", p=P)
    out_c = out_d.rearrange("(c p) d -> p c d", p=P)

    with tile.TileContext(nc) as tc, ExitStack() as ctx:
        big = ctx.enter_context(tc.tile_pool(name="big", bufs=1))
        ld = ctx.enter_context(tc.tile_pool(name="ld", bufs=2))
        sc_p = ctx.enter_context(tc.tile_pool(name="scr", bufs=2))
        xnp = ctx.enter_context(tc.tile_pool(name="xn", bufs=3))
        ep = ctx.enter_context(tc.tile_pool(name="exp", bufs=3))
        fin = ctx.enter_context(tc.tile_pool(name="fin", bufs=2))
        sp = ctx.enter_context(tc.tile_pool(name="spsum", bufs=2, space="PSUM"))
        pp = ctx.enter_context(tc.tile_pool(name="ppsum", bufs=1, space="PSUM"))

        # persistent tiles
        xa = big.tile([P, nch, 257], F16)          # raw x fp16 + ones col
        xnT0 = big.tile([P, n], F16)               # x_norm^T rows d=0..127
        xnT1 = big.tile([P, n], F16)               # x_norm^T rows d=128..255
        xnTm0 = big.tile([P, m], F16)              # core rows, normalized, transposed
        xnTm1 = big.tile([P, m], F16)
        xrs = big.tile([P, rch, D], F32)           # 1.1 * xr
        xrf = big.tile([P, rch, D], F16)           # fp16 copy of xr
        pacc = big.tile([P, nmt * msub, 257], F32)  # SBUF P/sumexp accumulators
        ssq_all = big.tile([P, nch], F32)
        rnorm_all = big.tile([P, nch], F32)
        ssq_r = big.tile([P, rch], F32)
        rnorm_r = big.tile([P, rch], F32)

        def xr_chain():
            """Core's own rows: xrs, xnTm (gates the very first matmul)."""
            for g in range(max(1, rch // G)):
                gw = min(G, rch)
                xt = ld.tile([P, G, D], F32, name=f"xtr{g}", tag="xt")
                nc.sync.dma_start(xt[:, 0:gw, :], xr_c[:, g * gw:(g + 1) * gw, :])
                nc.vector.tensor_scalar_mul(xrs[:, g * gw:(g + 1) * gw, :],
                                            xt[:, 0:gw, :], 1.1)
                for j in range(gw):
                    c = g * gw + j
                    scr = sc_p.tile([P, D], F16, tag="sq_scratch", name=f"scr_r{c}")
                    # row sum-of-squares via ACT Square + accum (TTR/tensor_reduce
                    # are broken on this runtime)
                    nc.scalar.activation(scr[:], xt[:, j, :], AF.Square,
                                         accum_out=ssq_r[:, c:c + 1])
                    nc.gpsimd.tensor_copy(xrf[:, c, :], xt[:, j, :])
                s = sc_p.tile([P, gw], F32, tag="nrm_scratch", name=f"s_r{g}")
                nc.scalar.sqrt(s[:], ssq_r[:, g * gw:(g + 1) * gw])
                nc.vector.reciprocal(rnorm_r[:, g * gw:(g + 1) * gw], s[:])
                for j in range(gw):
                    c = g * gw + j
                    xn = xnp.tile([P, D], F16, name=f"xnr{c}", tag="xn")
                    nc.vector.tensor_scalar_mul(xn[:], xrf[:, c, :],
                                                rnorm_r[:, c:c + 1])
                    # ACT is idle this early; give it the one-off xnTm1 set
                    nc.sync.dma_start_transpose(xnTm0[:, c * P:(c + 1) * P],
                                                xn[:, 0:P])
                    nc.scalar.dma_start_transpose(xnTm1[:, c * P:(c + 1) * P],
                                                  xn[:, P:D])

        def setup_group(g):
            """Load/convert/normalize/transpose chunks 4g..4g+3 of full x."""
            xt = ld.tile([P, G, D], F32, name=f"xt{g}", tag="xt")
            nc.sync.dma_start(xt[:], x_c[:, g * G:(g + 1) * G, :])
            for j in range(G):
                c = g * G + j
                scr = sc_p.tile([P, D], F16, tag="sq_scratch", name=f"scr{c}")
                nc.scalar.activation(scr[:], xt[:, j, :], AF.Square,
                                     accum_out=ssq_all[:, c:c + 1])
                # raw fp16 copy (gpsimd is otherwise idle) + ones column
                nc.gpsimd.tensor_copy(xa[:, c, 0:D], xt[:, j, :])
                nc.gpsimd.memset(xa[:, c, D:257], 1.0)
            s = sc_p.tile([P, G], F32, tag="nrm_scratch", name=f"s{g}")
            nc.scalar.sqrt(s[:], ssq_all[:, g * G:(g + 1) * G])
            nc.vector.reciprocal(rnorm_all[:, g * G:(g + 1) * G], s[:])
            for j in range(G):
                c = g * G + j
                xn = xnp.tile([P, D], F16, name=f"xn{c}", tag="xn")
                nc.vector.tensor_scalar_mul(xn[:], xa[:, c, 0:D],
                                            rnorm_all[:, c:c + 1])
                nc.sync.dma_start_transpose(xnT0[:, c * P:(c + 1) * P], xn[:, 0:P])
                nc.sync.dma_start_transpose(xnT1[:, c * P:(c + 1) * P], xn[:, P:D])

        def phase(ph):
            """All m-tiles consume chunks [ph*CP, (ph+1)*CP); flush into pacc."""
            for mt in range(nmt):
                m0 = mt * mt_w
                paug = [pp.tile([P, 257], F32, tag=f"paug{ms}",
                                name=f"paug{ms}_{ph}_{mt}") for ms in range(msub)]
                for scn in range(CP // 2):
                    c0 = ph * CP + scn * 2
                    stp = sp.tile([P, 2, mt_w], F32, name=f"stp{ph}_{mt}_{scn}",
                                  tag="stp")
                    for j in range(2):
                        c = c0 + j
                        nc.tensor.matmul(stp[:, j, :], xnT0[:, c * P:(c + 1) * P],
                                         xnTm0[:, m0:m0 + mt_w],
                                         start=True, stop=False)
                        nc.tensor.matmul(stp[:, j, :], xnT1[:, c * P:(c + 1) * P],
                                         xnTm1[:, m0:m0 + mt_w],
                                         start=False, stop=True)
                    es = ep.tile([P, 2, mt_w], F16, name=f"es{ph}_{mt}_{scn}",
                                 tag="es")
                    nc.scalar.activation(es[:], stp[:], AF.Exp)
                    for j in range(2):
                        c = c0 + j
                        first = (c == ph * CP)
                        last = (c == ph * CP + CP - 1)
                        for ms in range(msub):
                            nc.tensor.matmul(
                                paug[ms][:], es[:, j, ms * P:(ms + 1) * P],
                                xa[:, c, :], start=first, stop=last,
                            )
                for ms in range(msub):
                    acc = pacc[:, mt * msub + ms, :]
                    if ph == 0:
                        nc.vector.tensor_copy(acc, paug[ms][:])
                    else:
                        nc.vector.tensor_add(acc, acc, paug[ms][:])

        def finalize():
            for rc in range(nmt * msub):
                r = fin.tile([P, 1], F32, tag="recip", name=f"r{rc}")
                nc.vector.reciprocal(r[:], pacc[:, rc, 256:257])
                rs = fin.tile([P, 1], F32, tag="rscaled", name=f"rs{rc}")
                nc.vector.tensor_scalar_mul(rs[:], r[:], -SCALE)
                t1 = fin.tile([P, D], F32, tag="scaledP", name=f"t1{rc}")
                nc.vector.tensor_scalar_mul(t1[:], pacc[:, rc, 0:D], rs[:])
                ot = fin.tile([P, D], F32, tag="otile", name=f"ot{rc}")
                nc.vector.tensor_add(ot[:], xrs[:, rc, :], t1[:])
                nc.sync.dma_start(out_c[:, rc, :], ot[:])

        # ---- software-pipelined emission: setup stays LOOKAHEAD phases
        # ahead of compute so each engine's program order matches the
        # intended overlap ----
        xr_chain()
        emitted = 0
        prefill = min(ngrp, GPP * LOOKAHEAD)
        for g in range(prefill):
            setup_group(g)
            emitted += 1
        for ph in range(nph):
            want = min(ngrp, GPP * (ph + 1 + LOOKAHEAD))
            while emitted < want:
                setup_group(emitted)
                emitted += 1
            phase(ph)
        finalize()

    if compile:
        nc.compile()
    return nc


def _get_nc():
    global _NC
    if _NC is None:
        _NC = build()
    return _NC


def _run(x, trace=False):
    from concourse.bass_utils import run_bass_kernel_spmd

    x = np.ascontiguousarray(np.asarray(x, dtype=np.float32))
    assert x.shape == (N, D)
    in_maps = [{"x": x, "xr": x[c * M:(c + 1) * M]} for c in range(NCORES)]
    res = run_bass_kernel_spmd(_get_nc(), in_maps, core_ids=list(range(NCORES)),
                               trace=trace)
    out = np.concatenate([res.results[c]["out"] for c in range(NCORES)], axis=0)
    return out, res


def kernel(x):
    out, _ = _run(x, trace=False)
    return out
